# revision 6
# baseline (speedup 1.0000x reference)
"""Trainium2 Bass kernel for a cross-attention transformer layer.

Reference computation (per batch b):
    Q = query @ Wq.T + bq ; K = key @ Wk.T + bk ; V = value @ Wv.T + bv
    scores = QK^T/sqrt(d_k) per head, masked, softmax
    out = LayerNorm(softmax(scores) V @ Wo.T + bo + query)

Sharding: 8 cores = 4 batches x 2 query-halves. Each core computes the
full layer for its (batch, 1024-query-row) shard; K/V projections are
recomputed per half (no collectives needed). Output shards concatenate.

Device-side layout is the "transposed world": activations live as
[d_model, seq] (d on partitions) so projections, scores, attn*V and the
output projection chain into each other with no transposes:
  QT[d,q] = WqT.T @ queryT ;  KT[d,k] = WkT.T @ keyT   (head pairs share
     a 128-partition block: head h at partitions 64*(h%2), chunk h//2)
  V[k,d]  = valueT.T @ WvT                     (natural [k,d] layout)
  scoresT[k,q] = KT_h.T @ QT_h   row-packed: the two heads of a pair run
     concurrently on PE row strips (0,0)/(64,0) (64-dim contractions)
  PT[k,q] = exp(scoresT/8 + maskbias[k])  - one ACT op per [128,1024]
     PSUM tile; the mask rides in the per-partition bias AP
  ctx     = V_h.T @ PT  col-packed: pair heads write partitions 0:64 /
     64:128 of one PSUM tile via tile_position (0,0)/(0,64), fp32
     accumulation over the 16 k tiles
  sums    = ones.T @ PT  (M=1 matmuls col-packed to partition rows
     {0,32,64,96}; softmax denominators, same fp32 accumulation)
  Z[q,o]  = sum_do ctx[:,do].T @ WoT[:,do] + residual; LayerNorm over o.
"""

import sys

if "/opt/trn_rl_repo" not in sys.path:
    sys.path.insert(0, "/opt/trn_rl_repo")

import numpy as np
import ml_dtypes

import concourse.bacc as bacc
import concourse.mybir as mybir
import concourse.tile as tile
from concourse import bass_utils

F32 = mybir.dt.float32
BF16 = mybir.dt.bfloat16
AF = mybir.ActivationFunctionType
ALU = mybir.AluOpType

D_MODEL = 512
N_HEADS = 8
D_K = 64
SQ = 1024          # query rows per core
SK = 2048          # key rows per core
N_CORES = 8
P = 128
NEG = -1.0e9

_NC_CACHE: dict = {}


def _build(qkv_bias: bool, ln_affine: bool):
    """Build the per-core NEFF. All 8 cores run this same program."""
    nc = bacc.Bacc("TRN2", target_bir_lowering=False, debug=False,
                   enable_asserts=False, num_devices=N_CORES)

    d = lambda name, shape, dt: nc.dram_tensor(name, shape, dt, kind="ExternalInput").ap()
    qT = d("qT", [D_MODEL, SQ], BF16)
    kT = d("kT", [D_MODEL, SK], BF16)
    vT = d("vT", [D_MODEL, SK], BF16)
    qres = d("qres", [SQ, D_MODEL], F32)        # query rows + bo (residual)
    wqT = d("wqT", [D_MODEL, D_MODEL], BF16)    # Wq^T  [c_in, d_out]
    wkT = d("wkT", [D_MODEL, D_MODEL], BF16)
    wvT = d("wvT", [D_MODEL, D_MODEL], BF16)
    woT = d("woT", [D_MODEL, D_MODEL], BF16)
    maskbias = d("maskbias", [SK], F32)         # 0 / -1e9 per key
    if qkv_bias:
        bq = d("bq", [D_MODEL], F32)
        bk = d("bk", [D_MODEL], F32)
        bv = d("bv", [D_MODEL], F32)
    if ln_affine:
        gamma = d("gamma", [D_MODEL], F32)
        beta = d("beta", [D_MODEL], F32)
    out = nc.dram_tensor("out", [SQ, D_MODEL], F32, kind="ExternalOutput").ap()

    CO = D_MODEL // P   # 4 outer chunks of the model dim
    KT_TILES = SK // P  # 16 key tiles
    NPAIR = N_HEADS // 2

    with tile.TileContext(nc) as tc:
        with (
            tc.tile_pool(name="singles", bufs=1) as singles,
            tc.tile_pool(name="inbuf", bufs=1) as inbuf,
            tc.tile_pool(name="pt", bufs=8) as ptp,
            tc.tile_pool(name="small", bufs=4) as small,
            tc.tile_pool(name="stream", bufs=3) as stream,
            # PSUM budget (8 banks): sc 2x[128,1024]=4, ctx 2x[65,2,512]=4
            # (ctxA+ctxB).  proj/Z/rb matmuls share the "sc" slots.
            tc.tile_pool(name="ps_sc", bufs=2, space="PSUM") as ps_sc,
            tc.tile_pool(name="ps_ctx", bufs=1, space="PSUM") as ps_ctx,
        ):
            # ---- load weights + small params -------------------------------
            w_sb = {}
            for name, ap in (("wq", wqT), ("wk", wkT), ("wv", wvT), ("wo", woT)):
                t = singles.tile([P, CO, D_MODEL], BF16, tag=f"w_{name}")
                nc.sync.dma_start(t[:], ap.rearrange("(co ci) o -> ci co o", ci=P))
                w_sb[name] = t

            mb_sb = singles.tile([P, KT_TILES], F32, tag="mb")
            nc.sync.dma_start(mb_sb[:], maskbias.rearrange("(kt ki) -> ki kt", ki=P))

            if qkv_bias:
                bq_sb = singles.tile([P, CO], F32, tag="bq")
                nc.sync.dma_start(bq_sb[:], bq.rearrange("(co ci) -> ci co", ci=P))
                bk_sb = singles.tile([P, CO], F32, tag="bk")
                nc.sync.dma_start(bk_sb[:], bk.rearrange("(co ci) -> ci co", ci=P))
                bv_bc = singles.tile([P, D_MODEL], F32, tag="bv")
                nc.sync.dma_start(bv_bc[:], bv.to_broadcast((P, D_MODEL)))
            if ln_affine:
                gamma_bc = singles.tile([P, D_MODEL], F32, tag="gamma")
                nc.sync.dma_start(gamma_bc[:], gamma.to_broadcast((P, D_MODEL)))
                beta_bc = singles.tile([P, D_MODEL], F32, tag="beta")
                nc.sync.dma_start(beta_bc[:], beta.to_broadcast((P, D_MODEL)))
            eps_sb = singles.tile([P, 1], F32, tag="eps")
            nc.gpsimd.memset(eps_sb[:], 1e-5)
            ones1 = singles.tile([1, D_K], BF16, tag="ones1")
            nc.gpsimd.memset(ones1[:], 1.0)

            # ---- load activations (transposed layouts) ---------------------
            qT_sb = inbuf.tile([P, CO, SQ], BF16, tag="qT")
            nc.sync.dma_start(qT_sb[:], qT.rearrange("(co ci) q -> ci co q", ci=P))
            kT_sb = inbuf.tile([P, CO, SK], BF16, tag="kT")
            nc.sync.dma_start(kT_sb[:], kT.rearrange("(co ci) k -> ci co k", ci=P))
            vT_sb = inbuf.tile([P, CO, SK], BF16, tag="vT")
            nc.sync.dma_start(vT_sb[:], vT.rearrange("(co ci) k -> ci co k", ci=P))

            # ---- projections (psums ride in the "sc" slots) ----------------
            QT_sb = singles.tile([P, CO, SQ], BF16, tag="QT")
            for do in range(CO):
                for q0 in range(0, SQ, 512):
                    psq = ps_sc.tile([P, 512], F32, tag="sc")
                    for ci in range(CO):
                        nc.tensor.matmul(
                            psq[:], w_sb["wq"][:, ci, do * P:(do + 1) * P],
                            qT_sb[:, ci, q0:q0 + 512],
                            start=(ci == 0), stop=(ci == CO - 1))
                    dst = QT_sb[:, do, q0:q0 + 512]
                    if qkv_bias:
                        nc.vector.tensor_scalar_add(dst, psq[:], bq_sb[:, do:do + 1])
                    else:
                        nc.vector.tensor_copy(dst, psq[:])

            KT_sb = singles.tile([P, CO, SK], BF16, tag="KT")
            for do in range(CO):
                for k0 in range(0, SK, 512):
                    psk = ps_sc.tile([P, 512], F32, tag="sc")
                    for ci in range(CO):
                        nc.tensor.matmul(
                            psk[:], w_sb["wk"][:, ci, do * P:(do + 1) * P],
                            kT_sb[:, ci, k0:k0 + 512],
                            start=(ci == 0), stop=(ci == CO - 1))
                    dst = KT_sb[:, do, k0:k0 + 512]
                    if qkv_bias:
                        nc.vector.tensor_scalar_add(dst, psk[:], bk_sb[:, do:do + 1])
                    else:
                        nc.vector.tensor_copy(dst, psk[:])

            # V‖ones: 65-wide head slots; col 64 stays 1.0 from the memset so
            # the ctx matmul's 65th output row is the softmax denominator.
            V_sb = singles.tile([P, KT_TILES, N_HEADS, D_K + 1], BF16, tag="V")
            nc.gpsimd.memset(V_sb[:], 1.0)
            for st in range(KT_TILES):
                psv = ps_sc.tile([P, 512], F32, tag="sc")
                for ci in range(CO):
                    nc.tensor.matmul(
                        psv[:], vT_sb[:, ci, st * P:(st + 1) * P],
                        w_sb["wv"][:, ci, :],
                        start=(ci == 0), stop=(ci == CO - 1))
                dst = V_sb[:, st, :, 0:D_K]
                src = psv[:].rearrange("p (h e) -> p h e", h=N_HEADS)
                if qkv_bias:
                    nc.vector.tensor_tensor(
                        dst, src,
                        bv_bc[:].rearrange("p (h e) -> p h e", h=N_HEADS),
                        ALU.add)
                else:
                    nc.vector.tensor_copy(dst, src)

            # ---- attention: head pairs, full 1024-q tiles ------------------
            # ctx_sb[d, do, q] pair-major (matches O-proj lhsT layout)
            ctx_sb = singles.tile([P, CO, SQ], BF16, tag="ctx")
            # softmax denominators, row j = (head, q-chunk) = 2h + c
            sums_sb = singles.tile([2 * N_HEADS, 512], F32, tag="sums")

            for pair in range(NPAIR):
                hA, hB = 2 * pair, 2 * pair + 1
                do = pair
                # per-head [65, 2, 512] accumulators: rows 0:64 = V^T PT,
                # row 64 = ones^T PT (softmax denominator, via V‖ones)
                ctx_psA = ps_ctx.tile([D_K + 1, 2, 512], F32, tag="ctxA")
                ctx_psB = ps_ctx.tile([D_K + 1, 2, 512], F32, tag="ctxB")

                def consume(kt, ptA, ptB):
                    # ctx matmuls for tile kt (PT already computed)
                    first, last = kt == 0, kt == KT_TILES - 1
                    for c in range(2):
                        q0 = c * 512
                        nc.tensor.matmul(
                            ctx_psA[:, c, :], V_sb[:, kt, hA, :],
                            ptA[:, q0:q0 + 512], start=first, stop=last)
                        nc.tensor.matmul(
                            ctx_psB[:, c, :], V_sb[:, kt, hB, :],
                            ptB[:, q0:q0 + 512], start=first, stop=last)

                pending = None  # 1-tile software pipeline: PE consumes PT
                for kt in range(KT_TILES):  # while ACT works on the next one
                    k0 = kt * P
                    sA = ps_sc.tile([P, 1024], F32, tag="sc")
                    sB = ps_sc.tile([P, 1024], F32, tag="sc")
                    for c in range(2):
                        q0 = c * 512
                        nc.tensor.matmul(
                            sA[:, q0:q0 + 512],
                            KT_sb[0:D_K, do, k0:k0 + P],
                            QT_sb[0:D_K, do, q0:q0 + 512],
                            start=True, stop=True)
                        nc.tensor.matmul(
                            sB[:, q0:q0 + 512],
                            KT_sb[D_K:P, do, k0:k0 + P],
                            QT_sb[D_K:P, do, q0:q0 + 512],
                            start=True, stop=True)
                    ptA = ptp.tile([P, 1024], BF16, tag="pt")
                    nc.scalar.activation(ptA[:], sA[:], AF.Exp,
                                         bias=mb_sb[:, kt:kt + 1], scale=0.125)
                    ptB = ptp.tile([P, 1024], BF16, tag="pt")
                    nc.scalar.activation(ptB[:], sB[:], AF.Exp,
                                         bias=mb_sb[:, kt:kt + 1], scale=0.125)
                    if pending is not None:
                        consume(*pending)
                    pending = (kt, ptA, ptB)
                consume(*pending)
                # unnormalized context -> SBUF bf16 (normalized later);
                # head B lands on partitions 64:128 (partition-base shift)
                nc.vector.tensor_copy(
                    ctx_sb[0:D_K, do, :].rearrange("p (c q) -> p c q", c=2),
                    ctx_psA[0:D_K, :, :])
                nc.vector.tensor_copy(
                    ctx_sb[D_K:P, do, :].rearrange("p (c q) -> p c q", c=2),
                    ctx_psB[0:D_K, :, :])
                # denominators: psum row 64 -> SBUF (engine copy, same
                # partition), then DMA partition-moves into sums_sb rows
                sstA = small.tile([P, 2, 512], F32, tag="sstA")
                sstB = small.tile([P, 2, 512], F32, tag="sstB")
                nc.vector.tensor_copy(sstA[D_K:D_K + 1, :, :],
                                      ctx_psA[D_K:D_K + 1, :, :])
                nc.vector.tensor_copy(sstB[D_K:D_K + 1, :, :],
                                      ctx_psB[D_K:D_K + 1, :, :])
                for c in range(2):
                    nc.sync.dma_start(sums_sb[2 * hA + c:2 * hA + c + 1, :],
                                      sstA[D_K:D_K + 1, c, :])
                    nc.sync.dma_start(sums_sb[2 * hB + c:2 * hB + c + 1, :],
                                      sstB[D_K:D_K + 1, c, :])

            # ---- normalize context ----------------------------------------
            recip = singles.tile([2 * N_HEADS, 512], F32, tag="recip")
            nc.vector.reciprocal(recip[:], sums_sb[:])
            recip_bf = singles.tile([2 * N_HEADS, 512], BF16, tag="recipbf")
            nc.vector.tensor_copy(recip_bf[:], recip[:])
            # stage rows into partition 0's free dim, then broadcast each
            # across 64 PSUM partitions via 1-row PE outer products
            stage = singles.tile([1, 2 * N_HEADS, 512], BF16, tag="stg")
            for j in range(2 * N_HEADS):
                nc.sync.dma_start(stage[:, j, :], recip_bf[j:j + 1, :])
            for pair in range(NPAIR):
                hA, hB = 2 * pair, 2 * pair + 1
                for c in range(2):
                    rb = ps_sc.tile([P, 512], F32, tag="sc")
                    nc.tensor.matmul(rb[0:D_K, :], ones1[:],
                                     stage[:, 2 * hA + c, :],
                                     start=True, stop=True,
                                     tile_position=(0, 0),
                                     skip_group_check=True)
                    nc.tensor.matmul(rb[D_K:P, :], ones1[:],
                                     stage[:, 2 * hB + c, :],
                                     start=True, stop=True,
                                     tile_position=(0, D_K),
                                     skip_group_check=True)
                    cslice = ctx_sb[:, pair, c * 512:(c + 1) * 512]
                    nc.vector.tensor_tensor(cslice, cslice, rb[:], ALU.mult)

            # ---- output projection + residual + layernorm ------------------
            for qc in range(SQ // P):
                zps = ps_sc.tile([P, D_MODEL], F32, tag="sc")
                for do in range(CO):
                    nc.tensor.matmul(
                        zps[:], ctx_sb[:, do, qc * P:(qc + 1) * P],
                        w_sb["wo"][:, do, :],
                        start=(do == 0), stop=(do == CO - 1))
                qr = stream.tile([P, D_MODEL], F32, tag="qr")
                nc.sync.dma_start(qr[:], qres[qc * P:(qc + 1) * P, :])
                z = stream.tile([P, D_MODEL], F32, tag="z")
                nc.vector.tensor_tensor(z[:], zps[:], qr[:], ALU.add)
                stats = small.tile([P, 6], F32, tag="stats")
                nc.vector.bn_stats(stats[:], z[:])
                mv = small.tile([P, 2], F32, tag="mv")
                nc.vector.bn_aggr(mv[:], stats[:])
                istd = small.tile([P, 1], F32, tag="istd")
                nc.scalar.activation(istd[:], mv[:, 1:2], AF.Sqrt,
                                     bias=eps_sb[:], scale=1.0)
                nc.vector.reciprocal(istd[:], istd[:])
                zo = stream.tile([P, D_MODEL], F32, tag="zo")
                nc.vector.tensor_scalar(zo[:], z[:], mv[:, 0:1], istd[:],
                                        ALU.subtract, ALU.mult)
                if ln_affine:
                    nc.vector.tensor_tensor(zo[:], zo[:], gamma_bc[:], ALU.mult)
                    nc.vector.tensor_tensor(zo[:], zo[:], beta_bc[:], ALU.add)
                nc.sync.dma_start(out[qc * P:(qc + 1) * P, :], zo[:])

    nc.compile()
    return nc


def _get_nc(qkv_bias: bool, ln_affine: bool):
    key = (qkv_bias, ln_affine)
    if key not in _NC_CACHE:
        _NC_CACHE[key] = _build(*key)
    return _NC_CACHE[key]


def prepare(query, key, value, key_mask, Wq, bq, Wk, bk, Wv, bv, Wo, bo,
            ln_gamma, ln_beta):
    """Host-side prep: returns (nc, in_maps) for the 8 cores."""
    query = np.asarray(query, dtype=np.float32)
    key = np.asarray(key, dtype=np.float32)
    value = np.asarray(value, dtype=np.float32)
    key_mask = np.asarray(key_mask)
    Wq = np.asarray(Wq, dtype=np.float32)
    Wk = np.asarray(Wk, dtype=np.float32)
    Wv = np.asarray(Wv, dtype=np.float32)
    Wo = np.asarray(Wo, dtype=np.float32)
    bq = np.asarray(bq, dtype=np.float32)
    bk = np.asarray(bk, dtype=np.float32)
    bv = np.asarray(bv, dtype=np.float32)
    bo = np.asarray(bo, dtype=np.float32)
    ln_gamma = np.asarray(ln_gamma, dtype=np.float32)
    ln_beta = np.asarray(ln_beta, dtype=np.float32)

    B, sq_full, dm = query.shape
    assert (B, sq_full, dm) == (4, 2048, 512), query.shape

    qkv_bias = bool(bq.any() or bk.any() or bv.any())
    ln_affine = bool((ln_gamma != 1.0).any() or ln_beta.any())
    nc = _get_nc(qkv_bias, ln_affine)

    bf = ml_dtypes.bfloat16
    wqT = np.ascontiguousarray(Wq.T).astype(bf)
    wkT = np.ascontiguousarray(Wk.T).astype(bf)
    wvT = np.ascontiguousarray(Wv.T).astype(bf)
    woT = np.ascontiguousarray(Wo.T).astype(bf)
    maskbias = np.where(key_mask, np.float32(0.0), np.float32(NEG))
    qres_full = query + bo[None, None, :]

    in_maps = []
    for core in range(N_CORES):
        b, half = divmod(core, 2)
        rows = slice(half * SQ, (half + 1) * SQ)
        m = {
            "qT": np.ascontiguousarray(query[b, rows].T).astype(bf),
            "kT": np.ascontiguousarray(key[b].T).astype(bf),
            "vT": np.ascontiguousarray(value[b].T).astype(bf),
            "qres": np.ascontiguousarray(qres_full[b, rows]),
            "wqT": wqT, "wkT": wkT, "wvT": wvT, "woT": woT,
            "maskbias": np.ascontiguousarray(maskbias[b]),
        }
        if qkv_bias:
            m["bq"], m["bk"], m["bv"] = bq, bk, bv
        if ln_affine:
            m["gamma"], m["beta"] = ln_gamma, ln_beta
        in_maps.append(m)
    return nc, in_maps


def kernel(**inputs):
    nc, in_maps = prepare(**inputs)
    B, sq_full, dm = 4, 2048, 512

    res = bass_utils.run_bass_kernel_spmd(nc, in_maps,
                                          core_ids=list(range(N_CORES)))
    out = np.empty((B, sq_full, dm), dtype=np.float32)
    for core in range(N_CORES):
        b, half = divmod(core, 2)
        out[b, half * SQ:(half + 1) * SQ] = res.results[core]["out"]
    return out



# revision 12
# speedup vs baseline: 1.0059x; 1.0059x over previous
"""Trainium2 Bass kernel for a cross-attention transformer layer.

Reference computation (per batch b):
    Q = query @ Wq.T + bq ; K = key @ Wk.T + bk ; V = value @ Wv.T + bv
    scores = QK^T/sqrt(d_k) per head, masked, softmax
    out = LayerNorm(softmax(scores) V @ Wo.T + bo + query)

Sharding: 8 cores = 4 batches x 2 query-halves. Each core computes the
full layer for its (batch, 1024-query-row) shard; K/V projections are
recomputed per half (no collectives needed). Output shards concatenate.

Device-side layout is the "transposed world": activations live as
[d_model, seq] (d on partitions) so projections, scores, attn*V and the
output projection chain into each other with no transposes:
  QT[d,q] = WqT.T @ queryT ;  KT[d,k] = WkT.T @ keyT   (head pairs share
     a 128-partition block: head h at partitions 64*(h%2), chunk h//2)
  V[k,d]  = valueT.T @ WvT                     (natural [k,d] layout)
  scoresT[k,q] = KT_h.T @ QT_h   row-packed: the two heads of a pair run
     concurrently on PE row strips (0,0)/(64,0) (64-dim contractions)
  PT[k,q] = exp(scoresT/8 + maskbias[k])  - one ACT op per [128,1024]
     PSUM tile; the mask rides in the per-partition bias AP
  ctx     = V_h.T @ PT  col-packed: pair heads write partitions 0:64 /
     64:128 of one PSUM tile via tile_position (0,0)/(0,64), fp32
     accumulation over the 16 k tiles
  sums    = ones.T @ PT  (M=1 matmuls col-packed to partition rows
     {0,32,64,96}; softmax denominators, same fp32 accumulation)
  Z[q,o]  = sum_do ctx[:,do].T @ WoT[:,do] + residual; LayerNorm over o.
"""

import sys

if "/opt/trn_rl_repo" not in sys.path:
    sys.path.insert(0, "/opt/trn_rl_repo")

import numpy as np
import ml_dtypes

import concourse.bacc as bacc
import concourse.mybir as mybir
import concourse.tile as tile
from concourse import bass_utils

F32 = mybir.dt.float32
BF16 = mybir.dt.bfloat16
F8 = mybir.dt.float8e4
AF = mybir.ActivationFunctionType
ALU = mybir.AluOpType
DR = mybir.MatmulPerfMode.DoubleRow

D_MODEL = 512
N_HEADS = 8
D_K = 64
SQ = 1024          # query rows per core
SK = 2048          # key rows per core
N_CORES = 8
P = 128
NEG = -1.0e9

_NC_CACHE: dict = {}


def _build(qkv_bias: bool, ln_affine: bool):
    """Build the per-core NEFF. All 8 cores run this same program."""
    nc = bacc.Bacc("TRN2", target_bir_lowering=False, debug=False,
                   enable_asserts=False, num_devices=N_CORES)

    d = lambda name, shape, dt: nc.dram_tensor(name, shape, dt, kind="ExternalInput").ap()
    qT = d("qT", [D_MODEL, SQ], BF16)
    kT = d("kT", [D_MODEL, SK], BF16)
    vT = d("vT", [D_MODEL, SK], BF16)
    qres = d("qres", [SQ, D_MODEL], F32)        # query rows + bo (residual)
    wqT = d("wqT", [D_MODEL, D_MODEL], BF16)    # Wq^T  [c_in, d_out]
    wkT = d("wkT", [D_MODEL, D_MODEL], BF16)
    wvT = d("wvT", [D_MODEL, D_MODEL], BF16)
    woT = d("woT", [D_MODEL, D_MODEL], BF16)
    maskbias = d("maskbias", [SK], F32)         # 0 / -1e9 per key
    if qkv_bias:
        bq = d("bq", [D_MODEL], F32)
        bk = d("bk", [D_MODEL], F32)
        bv = d("bv", [D_MODEL], F32)
    if ln_affine:
        gamma = d("gamma", [D_MODEL], F32)
        beta = d("beta", [D_MODEL], F32)
    out = nc.dram_tensor("out", [SQ, D_MODEL], F32, kind="ExternalOutput").ap()

    CO = D_MODEL // P   # 4 outer chunks of the model dim
    KT_TILES = SK // P  # 16 key tiles
    NPAIR = N_HEADS // 2

    with tile.TileContext(nc) as tc:
        with (
            tc.tile_pool(name="singles", bufs=1) as singles,
            tc.tile_pool(name="inbuf", bufs=1) as inbuf,
            tc.tile_pool(name="pt", bufs=2) as ptp,
            tc.tile_pool(name="small", bufs=4) as small,
            tc.tile_pool(name="stream", bufs=3) as stream,
            # PSUM budget (8 banks): sc 2x[128,1024]=4, ctx 2x[65,2,512]=4
            # (ctxA+ctxB).  proj/Z/rb matmuls share the "sc" slots.
            tc.tile_pool(name="ps_sc", bufs=2, space="PSUM") as ps_sc,
            tc.tile_pool(name="ps_ctx", bufs=1, space="PSUM") as ps_ctx,
        ):
            # ---- load weights + small params -------------------------------
            w_sb = {}
            for name, ap in (("wq", wqT), ("wk", wkT), ("wv", wvT), ("wo", woT)):
                t = singles.tile([P, CO, D_MODEL], BF16, tag=f"w_{name}")
                nc.sync.dma_start(t[:], ap.rearrange("(co ci) o -> ci co o", ci=P))
                w_sb[name] = t

            mb_sb = singles.tile([P, KT_TILES], F32, tag="mb")
            nc.sync.dma_start(mb_sb[:], maskbias.rearrange("(kt ki) -> ki kt", ki=P))

            if qkv_bias:
                bq_sb = singles.tile([P, CO], F32, tag="bq")
                nc.sync.dma_start(bq_sb[:], bq.rearrange("(co ci) -> ci co", ci=P))
                bk_sb = singles.tile([P, CO], F32, tag="bk")
                nc.sync.dma_start(bk_sb[:], bk.rearrange("(co ci) -> ci co", ci=P))
                bv_bc = singles.tile([P, D_MODEL], F32, tag="bv")
                nc.sync.dma_start(bv_bc[:], bv.to_broadcast((P, D_MODEL)))
            if ln_affine:
                gamma_bc = singles.tile([P, D_MODEL], F32, tag="gamma")
                nc.sync.dma_start(gamma_bc[:], gamma.to_broadcast((P, D_MODEL)))
                beta_bc = singles.tile([P, D_MODEL], F32, tag="beta")
                nc.sync.dma_start(beta_bc[:], beta.to_broadcast((P, D_MODEL)))
            eps_sb = singles.tile([P, 1], F32, tag="eps")
            nc.gpsimd.memset(eps_sb[:], 1e-5)
            ones1 = singles.tile([1, D_K], BF16, tag="ones1")
            nc.gpsimd.memset(ones1[:], 1.0)

            # ---- load activations (transposed layouts) ---------------------
            qT_sb = inbuf.tile([P, CO, SQ], BF16, tag="qT")
            nc.sync.dma_start(qT_sb[:], qT.rearrange("(co ci) q -> ci co q", ci=P))
            kT_sb = inbuf.tile([P, CO, SK], BF16, tag="kT")
            nc.sync.dma_start(kT_sb[:], kT.rearrange("(co ci) k -> ci co k", ci=P))
            vT_sb = inbuf.tile([P, CO, SK], BF16, tag="vT")
            nc.sync.dma_start(vT_sb[:], vT.rearrange("(co ci) k -> ci co k", ci=P))

            # ---- projections (psums ride in the "sc" slots) ----------------
            QT_sb = singles.tile([P, CO, SQ], BF16, tag="QT")
            for do in range(CO):
                for q0 in range(0, SQ, 512):
                    psq = ps_sc.tile([P, 512], F32, tag="sc")
                    for ci in range(CO):
                        nc.tensor.matmul(
                            psq[:], w_sb["wq"][:, ci, do * P:(do + 1) * P],
                            qT_sb[:, ci, q0:q0 + 512],
                            start=(ci == 0), stop=(ci == CO - 1))
                    dst = QT_sb[:, do, q0:q0 + 512]
                    if qkv_bias:
                        nc.vector.tensor_scalar_add(dst, psq[:], bq_sb[:, do:do + 1])
                    else:
                        nc.vector.tensor_copy(dst, psq[:])

            KT_sb = singles.tile([P, CO, SK], BF16, tag="KT")
            for do in range(CO):
                for k0 in range(0, SK, 512):
                    psk = ps_sc.tile([P, 512], F32, tag="sc")
                    for ci in range(CO):
                        nc.tensor.matmul(
                            psk[:], w_sb["wk"][:, ci, do * P:(do + 1) * P],
                            kT_sb[:, ci, k0:k0 + 512],
                            start=(ci == 0), stop=(ci == CO - 1))
                    dst = KT_sb[:, do, k0:k0 + 512]
                    if qkv_bias:
                        nc.vector.tensor_scalar_add(dst, psk[:], bk_sb[:, do:do + 1])
                    else:
                        nc.vector.tensor_copy(dst, psk[:])

            # V‖ones: 65-wide head slots; col 64 stays 1.0 from the memset so
            # the ctx matmul's 65th output row is the softmax denominator.
            # fp8 so the ctx matmul runs in DoubleRow (2 k-tiles/instruction).
            # head slot padded to 66 so the kt-pair stride (8*66 B) is a
            # multiple of 16 (dual-fp8 LdWeights AP restriction)
            V_sb = singles.tile([P, KT_TILES, N_HEADS, D_K + 2], F8, tag="V")
            nc.gpsimd.memset(V_sb[:], 1.0)
            for st in range(KT_TILES):
                psv = ps_sc.tile([P, 512], F32, tag="sc")
                for ci in range(CO):
                    nc.tensor.matmul(
                        psv[:], vT_sb[:, ci, st * P:(st + 1) * P],
                        w_sb["wv"][:, ci, :],
                        start=(ci == 0), stop=(ci == CO - 1))
                dst = V_sb[:, st, :, 0:D_K]
                src = psv[:].rearrange("p (h e) -> p h e", h=N_HEADS)
                if qkv_bias:
                    nc.vector.tensor_tensor(
                        dst, src,
                        bv_bc[:].rearrange("p (h e) -> p h e", h=N_HEADS),
                        ALU.add)
                else:
                    nc.vector.tensor_copy(dst, src)

            # ---- attention: head pairs, full 1024-q tiles ------------------
            # ctx_sb[d, do, q] pair-major (matches O-proj lhsT layout)
            ctx_sb = singles.tile([P, CO, SQ], BF16, tag="ctx")
            # softmax denominators, row j = (head, q-chunk) = 2h + c
            sums_sb = singles.tile([2 * N_HEADS, 512], F32, tag="sums")

            for pair in range(NPAIR):
                hA, hB = 2 * pair, 2 * pair + 1
                do = pair
                # per-head [65, 2, 512] accumulators: rows 0:64 = V^T PT,
                # row 64 = ones^T PT (softmax denominator, via V‖ones)
                ctx_psA = ps_ctx.tile([D_K + 1, 2, 512], F32, tag="ctxA")
                ctx_psB = ps_ctx.tile([D_K + 1, 2, 512], F32, tag="ctxB")

                def consume(t, ptA, ptB):
                    # DoubleRow ctx matmuls for k-tile pair t (PT computed):
                    # each instruction contracts both k-tiles of the pair
                    first, last = t == 0, t == KT_TILES // 2 - 1
                    for c in range(2):
                        q0 = c * 512
                        nc.tensor.matmul(
                            ctx_psA[:, c, :],
                            V_sb[:, 2 * t:2 * t + 2, hA, 0:D_K + 1],
                            ptA[:, :, q0:q0 + 512], start=first, stop=last,
                            perf_mode=DR)
                        nc.tensor.matmul(
                            ctx_psB[:, c, :],
                            V_sb[:, 2 * t:2 * t + 2, hB, 0:D_K + 1],
                            ptB[:, :, q0:q0 + 512], start=first, stop=last,
                            perf_mode=DR)

                pending = None  # 1-pair software pipeline: PE consumes PT
                for t in range(KT_TILES // 2):  # while ACT fills the next one
                    ptA = ptp.tile([P, 2, 1024], F8, tag="ptA")
                    ptB = ptp.tile([P, 2, 1024], F8, tag="ptB")
                    for i in range(2):
                        kt = 2 * t + i
                        k0 = kt * P
                        sA = ps_sc.tile([P, 1024], F32, tag="sc")
                        sB = ps_sc.tile([P, 1024], F32, tag="sc")
                        for c in range(2):
                            q0 = c * 512
                            nc.tensor.matmul(
                                sA[:, q0:q0 + 512],
                                KT_sb[0:D_K, do, k0:k0 + P],
                                QT_sb[0:D_K, do, q0:q0 + 512],
                                start=True, stop=True)
                            nc.tensor.matmul(
                                sB[:, q0:q0 + 512],
                                KT_sb[D_K:P, do, k0:k0 + P],
                                QT_sb[D_K:P, do, q0:q0 + 512],
                                start=True, stop=True)
                        nc.scalar.activation(ptA[:, i, :], sA[:], AF.Exp,
                                             bias=mb_sb[:, kt:kt + 1], scale=0.125)
                        nc.scalar.activation(ptB[:, i, :], sB[:], AF.Exp,
                                             bias=mb_sb[:, kt:kt + 1], scale=0.125)
                    if pending is not None:
                        consume(*pending)
                    pending = (t, ptA, ptB)
                consume(*pending)
                # unnormalized context -> SBUF bf16 (normalized later);
                # head B lands on partitions 64:128 (partition-base shift)
                nc.vector.tensor_copy(
                    ctx_sb[0:D_K, do, :].rearrange("p (c q) -> p c q", c=2),
                    ctx_psA[0:D_K, :, :])
                nc.vector.tensor_copy(
                    ctx_sb[D_K:P, do, :].rearrange("p (c q) -> p c q", c=2),
                    ctx_psB[0:D_K, :, :])
                # denominators: psum row 64 -> SBUF (engine copy, same
                # partition), then DMA partition-moves into sums_sb rows
                sstA = small.tile([P, 2, 512], F32, tag="sstA")
                sstB = small.tile([P, 2, 512], F32, tag="sstB")
                nc.vector.tensor_copy(sstA[D_K:D_K + 1, :, :],
                                      ctx_psA[D_K:D_K + 1, :, :])
                nc.vector.tensor_copy(sstB[D_K:D_K + 1, :, :],
                                      ctx_psB[D_K:D_K + 1, :, :])
                for c in range(2):
                    nc.sync.dma_start(sums_sb[2 * hA + c:2 * hA + c + 1, :],
                                      sstA[D_K:D_K + 1, c, :])
                    nc.sync.dma_start(sums_sb[2 * hB + c:2 * hB + c + 1, :],
                                      sstB[D_K:D_K + 1, c, :])

            # ---- normalize context ----------------------------------------
            recip = singles.tile([2 * N_HEADS, 512], F32, tag="recip")
            nc.vector.reciprocal(recip[:], sums_sb[:])
            recip_bf = singles.tile([2 * N_HEADS, 512], BF16, tag="recipbf")
            nc.vector.tensor_copy(recip_bf[:], recip[:])
            # stage rows into partition 0's free dim, then broadcast each
            # across 64 PSUM partitions via 1-row PE outer products
            stage = singles.tile([1, 2 * N_HEADS, 512], BF16, tag="stg")
            for j in range(2 * N_HEADS):
                nc.sync.dma_start(stage[:, j, :], recip_bf[j:j + 1, :])
            for pair in range(NPAIR):
                hA, hB = 2 * pair, 2 * pair + 1
                for c in range(2):
                    rb = ps_sc.tile([P, 512], F32, tag="sc")
                    nc.tensor.matmul(rb[0:D_K, :], ones1[:],
                                     stage[:, 2 * hA + c, :],
                                     start=True, stop=True,
                                     tile_position=(0, 0),
                                     skip_group_check=True)
                    nc.tensor.matmul(rb[D_K:P, :], ones1[:],
                                     stage[:, 2 * hB + c, :],
                                     start=True, stop=True,
                                     tile_position=(0, D_K),
                                     skip_group_check=True)
                    cslice = ctx_sb[:, pair, c * 512:(c + 1) * 512]
                    nc.vector.tensor_tensor(cslice, cslice, rb[:], ALU.mult)

            # ---- output projection + residual + layernorm ------------------
            for qc in range(SQ // P):
                zps = ps_sc.tile([P, D_MODEL], F32, tag="sc")
                for do in range(CO):
                    nc.tensor.matmul(
                        zps[:], ctx_sb[:, do, qc * P:(qc + 1) * P],
                        w_sb["wo"][:, do, :],
                        start=(do == 0), stop=(do == CO - 1))
                qr = stream.tile([P, D_MODEL], F32, tag="qr")
                nc.sync.dma_start(qr[:], qres[qc * P:(qc + 1) * P, :])
                z = stream.tile([P, D_MODEL], F32, tag="z")
                nc.vector.tensor_tensor(z[:], zps[:], qr[:], ALU.add)
                stats = small.tile([P, 6], F32, tag="stats")
                nc.vector.bn_stats(stats[:], z[:])
                mv = small.tile([P, 2], F32, tag="mv")
                nc.vector.bn_aggr(mv[:], stats[:])
                istd = small.tile([P, 1], F32, tag="istd")
                nc.scalar.activation(istd[:], mv[:, 1:2], AF.Sqrt,
                                     bias=eps_sb[:], scale=1.0)
                nc.vector.reciprocal(istd[:], istd[:])
                zo = stream.tile([P, D_MODEL], F32, tag="zo")
                nc.vector.tensor_scalar(zo[:], z[:], mv[:, 0:1], istd[:],
                                        ALU.subtract, ALU.mult)
                if ln_affine:
                    nc.vector.tensor_tensor(zo[:], zo[:], gamma_bc[:], ALU.mult)
                    nc.vector.tensor_tensor(zo[:], zo[:], beta_bc[:], ALU.add)
                nc.sync.dma_start(out[qc * P:(qc + 1) * P, :], zo[:])

    nc.compile()
    return nc


def _get_nc(qkv_bias: bool, ln_affine: bool):
    key = (qkv_bias, ln_affine)
    if key not in _NC_CACHE:
        _NC_CACHE[key] = _build(*key)
    return _NC_CACHE[key]


def prepare(query, key, value, key_mask, Wq, bq, Wk, bk, Wv, bv, Wo, bo,
            ln_gamma, ln_beta):
    """Host-side prep: returns (nc, in_maps) for the 8 cores."""
    query = np.asarray(query, dtype=np.float32)
    key = np.asarray(key, dtype=np.float32)
    value = np.asarray(value, dtype=np.float32)
    key_mask = np.asarray(key_mask)
    Wq = np.asarray(Wq, dtype=np.float32)
    Wk = np.asarray(Wk, dtype=np.float32)
    Wv = np.asarray(Wv, dtype=np.float32)
    Wo = np.asarray(Wo, dtype=np.float32)
    bq = np.asarray(bq, dtype=np.float32)
    bk = np.asarray(bk, dtype=np.float32)
    bv = np.asarray(bv, dtype=np.float32)
    bo = np.asarray(bo, dtype=np.float32)
    ln_gamma = np.asarray(ln_gamma, dtype=np.float32)
    ln_beta = np.asarray(ln_beta, dtype=np.float32)

    B, sq_full, dm = query.shape
    assert (B, sq_full, dm) == (4, 2048, 512), query.shape

    qkv_bias = bool(bq.any() or bk.any() or bv.any())
    ln_affine = bool((ln_gamma != 1.0).any() or ln_beta.any())
    nc = _get_nc(qkv_bias, ln_affine)

    bf = ml_dtypes.bfloat16
    wqT = np.ascontiguousarray(Wq.T).astype(bf)
    wkT = np.ascontiguousarray(Wk.T).astype(bf)
    wvT = np.ascontiguousarray(Wv.T).astype(bf)
    woT = np.ascontiguousarray(Wo.T).astype(bf)
    maskbias = np.where(key_mask, np.float32(0.0), np.float32(NEG))
    qres_full = query + bo[None, None, :]

    in_maps = []
    for core in range(N_CORES):
        b, half = divmod(core, 2)
        rows = slice(half * SQ, (half + 1) * SQ)
        m = {
            "qT": np.ascontiguousarray(query[b, rows].T).astype(bf),
            "kT": np.ascontiguousarray(key[b].T).astype(bf),
            "vT": np.ascontiguousarray(value[b].T).astype(bf),
            "qres": np.ascontiguousarray(qres_full[b, rows]),
            "wqT": wqT, "wkT": wkT, "wvT": wvT, "woT": woT,
            "maskbias": np.ascontiguousarray(maskbias[b]),
        }
        if qkv_bias:
            m["bq"], m["bk"], m["bv"] = bq, bk, bv
        if ln_affine:
            m["gamma"], m["beta"] = ln_gamma, ln_beta
        in_maps.append(m)
    return nc, in_maps


def kernel(**inputs):
    nc, in_maps = prepare(**inputs)
    B, sq_full, dm = 4, 2048, 512

    res = bass_utils.run_bass_kernel_spmd(nc, in_maps,
                                          core_ids=list(range(N_CORES)))
    out = np.empty((B, sq_full, dm), dtype=np.float32)
    for core in range(N_CORES):
        b, half = divmod(core, 2)
        out[b, half * SQ:(half + 1) * SQ] = res.results[core]["out"]
    return out



# revision 17
# speedup vs baseline: 1.0355x; 1.0294x over previous
"""Trainium2 Bass kernel for a cross-attention transformer layer.

Reference computation (per batch b):
    Q = query @ Wq.T + bq ; K = key @ Wk.T + bk ; V = value @ Wv.T + bv
    scores = QK^T/sqrt(d_k) per head, masked, softmax
    out = LayerNorm(softmax(scores) V @ Wo.T + bo + query)

Sharding: 8 cores = 4 batches x 2 query-halves. Each core computes the
full layer for its (batch, 1024-query-row) shard; K/V projections are
recomputed per half (no collectives needed). Output shards concatenate.

Device-side layout is the "transposed world": activations live as
[d_model, seq] (d on partitions) so projections, scores, attn*V and the
output projection chain into each other with no transposes:
  QT[d,q] = WqT.T @ queryT ;  KT[d,k] = WkT.T @ keyT   (head pairs share
     a 128-partition block: head h at partitions 64*(h%2), chunk h//2)
  V[k,d]  = valueT.T @ WvT                     (natural [k,d] layout)
  scoresT[k,q] = KT_h.T @ QT_h   row-packed: the two heads of a pair run
     concurrently on PE row strips (0,0)/(64,0) (64-dim contractions)
  PT[k,q] = exp(scoresT/8 + maskbias[k])  - one ACT op per [128,1024]
     PSUM tile; the mask rides in the per-partition bias AP
  ctx     = V_h.T @ PT  col-packed: pair heads write partitions 0:64 /
     64:128 of one PSUM tile via tile_position (0,0)/(0,64), fp32
     accumulation over the 16 k tiles
  sums    = ones.T @ PT  (M=1 matmuls col-packed to partition rows
     {0,32,64,96}; softmax denominators, same fp32 accumulation)
  Z[q,o]  = sum_do ctx[:,do].T @ WoT[:,do] + residual; LayerNorm over o.
"""

import sys

if "/opt/trn_rl_repo" not in sys.path:
    sys.path.insert(0, "/opt/trn_rl_repo")

import numpy as np
import ml_dtypes

import concourse.bacc as bacc
import concourse.mybir as mybir
import concourse.tile as tile
from concourse import bass_utils

F32 = mybir.dt.float32
BF16 = mybir.dt.bfloat16
F8 = mybir.dt.float8e4
AF = mybir.ActivationFunctionType
ALU = mybir.AluOpType
DR = mybir.MatmulPerfMode.DoubleRow

D_MODEL = 512
N_HEADS = 8
D_K = 64
SQ = 1024          # query rows per core
SK = 2048          # key rows per core
N_CORES = 8
P = 128
NEG = -1.0e9

_NC_CACHE: dict = {}


def _build(qkv_bias: bool, ln_affine: bool):
    """Build the per-core NEFF. All 8 cores run this same program."""
    nc = bacc.Bacc("TRN2", target_bir_lowering=False, debug=False,
                   enable_asserts=False, num_devices=N_CORES)

    d = lambda name, shape, dt: nc.dram_tensor(name, shape, dt, kind="ExternalInput").ap()
    qT = d("qT", [D_MODEL, SQ], BF16)
    kT = d("kT", [D_MODEL, SK], BF16)
    vT = d("vT", [D_MODEL, SK], BF16)
    qres = d("qres", [SQ, D_MODEL], F32)        # query rows + bo (residual)
    wqT = d("wqT", [D_MODEL, D_MODEL], BF16)    # Wq^T  [c_in, d_out]
    wkT = d("wkT", [D_MODEL, D_MODEL], BF16)
    wvT = d("wvT", [D_MODEL, D_MODEL], BF16)
    woT = d("woT", [D_MODEL, D_MODEL], BF16)
    maskbias = d("maskbias", [SK], F32)         # 0 / -1e9 per key
    if qkv_bias:
        bq = d("bq", [D_MODEL], F32)
        bk = d("bk", [D_MODEL], F32)
        bv = d("bv", [D_MODEL], F32)
    if ln_affine:
        gamma = d("gamma", [D_MODEL], F32)
        beta = d("beta", [D_MODEL], F32)
    out = nc.dram_tensor("out", [SQ, D_MODEL], F32, kind="ExternalOutput").ap()

    CO = D_MODEL // P   # 4 outer chunks of the model dim
    KT_TILES = SK // P  # 16 key tiles
    NPAIR = N_HEADS // 2

    with tile.TileContext(nc) as tc:
        with (
            tc.tile_pool(name="singles", bufs=1) as singles,
            tc.tile_pool(name="inbuf", bufs=1) as inbuf,
            tc.tile_pool(name="pt", bufs=3) as ptp,
            tc.tile_pool(name="small", bufs=4) as small,
            tc.tile_pool(name="stream", bufs=3) as stream,
            # PSUM budget (8 banks): sc 2x[128,1024]=4, ctx 2x[65,2,512]=4
            # (ctxA+ctxB).  proj/Z/rb matmuls share the "sc" slots.
            tc.tile_pool(name="ps_sc", bufs=2, space="PSUM") as ps_sc,
            tc.tile_pool(name="ps_ctx", bufs=1, space="PSUM") as ps_ctx,
        ):
            # ---- DMAs, ordered so the pair-0 critical path loads first -----
            w_sb = {name: singles.tile([P, CO, D_MODEL], BF16, tag=f"w_{name}",
                                       name=f"w_{name}")
                    for name in ("wq", "wk", "wv", "wo")}
            kT_sb = inbuf.tile([P, CO, SK], BF16, tag="kT")
            qT_sb = inbuf.tile([P, CO, SQ], BF16, tag="qT")
            vT_sb = inbuf.tile([P, CO, SK], BF16, tag="vT")
            mb_sb = singles.tile([P, KT_TILES], F32, tag="mb")
            qres_sb = singles.tile([P, SQ // P, D_MODEL], F32, tag="qres")

            nc.sync.dma_start(w_sb["wk"][:],
                              wkT.rearrange("(co ci) o -> ci co o", ci=P))
            nc.sync.dma_start(kT_sb[:], kT.rearrange("(co ci) k -> ci co k", ci=P))
            nc.sync.dma_start(w_sb["wq"][:],
                              wqT.rearrange("(co ci) o -> ci co o", ci=P))
            nc.sync.dma_start(qT_sb[:], qT.rearrange("(co ci) q -> ci co q", ci=P))
            nc.sync.dma_start(mb_sb[:], maskbias.rearrange("(kt ki) -> ki kt", ki=P))
            nc.sync.dma_start(w_sb["wv"][:],
                              wvT.rearrange("(co ci) o -> ci co o", ci=P))
            vT_re = vT.rearrange("(co ci) k -> ci co k", ci=P)
            for j in range(4):  # by key range: V proj tile st needs chunk st//4
                nc.sync.dma_start(vT_sb[:, :, j * 512:(j + 1) * 512],
                                  vT_re[:, :, j * 512:(j + 1) * 512])
            if qkv_bias:
                bq_sb = singles.tile([P, CO], F32, tag="bq")
                nc.sync.dma_start(bq_sb[:], bq.rearrange("(co ci) -> ci co", ci=P))
                bk_sb = singles.tile([P, CO], F32, tag="bk")
                nc.sync.dma_start(bk_sb[:], bk.rearrange("(co ci) -> ci co", ci=P))
                bv_bc = singles.tile([P, D_MODEL], F32, tag="bv")
                nc.sync.dma_start(bv_bc[:], bv.to_broadcast((P, D_MODEL)))
            nc.sync.dma_start(w_sb["wo"][:],
                              woT.rearrange("(co ci) o -> ci co o", ci=P))
            nc.sync.dma_start(qres_sb[:],
                              qres.rearrange("(qc p) o -> p qc o", p=P))
            if ln_affine:
                gamma_bc = singles.tile([P, D_MODEL], F32, tag="gamma")
                nc.sync.dma_start(gamma_bc[:], gamma.to_broadcast((P, D_MODEL)))
                beta_bc = singles.tile([P, D_MODEL], F32, tag="beta")
                nc.sync.dma_start(beta_bc[:], beta.to_broadcast((P, D_MODEL)))
            eps_sb = singles.tile([P, 1], F32, tag="eps")
            nc.gpsimd.memset(eps_sb[:], 1e-5)
            ones1 = singles.tile([1, D_K], BF16, tag="ones1")
            nc.gpsimd.memset(ones1[:], 1.0)

            # ---- projection emitters (psums ride in the "sc" slots) --------
            QT_sb = singles.tile([P, CO, SQ], BF16, tag="QT")
            KT_sb = singles.tile([P, CO, SK], BF16, tag="KT")
            # V‖ones: fp8 66-wide head slots (66*8 B kt-pair stride satisfies
            # the dual-fp8 LdWeights AP rule); col 64 = 1.0 makes the ctx
            # matmul's 65th output row the softmax denominator.
            V_sb = singles.tile([P, KT_TILES, N_HEADS, D_K + 2], F8, tag="V")
            nc.gpsimd.memset(V_sb[:, :, :, D_K:D_K + 2], 1.0)

            def proj_q(do, q0):
                psq = ps_sc.tile([P, 512], F32, tag="sc")
                for ci in range(CO):
                    nc.tensor.matmul(
                        psq[:], w_sb["wq"][:, ci, do * P:(do + 1) * P],
                        qT_sb[:, ci, q0:q0 + 512],
                        start=(ci == 0), stop=(ci == CO - 1))
                dst = QT_sb[:, do, q0:q0 + 512]
                if qkv_bias:
                    nc.vector.tensor_scalar_add(dst, psq[:], bq_sb[:, do:do + 1])
                else:
                    nc.vector.tensor_copy(dst, psq[:])

            def proj_k(do, k0):
                psk = ps_sc.tile([P, 512], F32, tag="sc")
                for ci in range(CO):
                    nc.tensor.matmul(
                        psk[:], w_sb["wk"][:, ci, do * P:(do + 1) * P],
                        kT_sb[:, ci, k0:k0 + 512],
                        start=(ci == 0), stop=(ci == CO - 1))
                dst = KT_sb[:, do, k0:k0 + 512]
                if qkv_bias:
                    nc.vector.tensor_scalar_add(dst, psk[:], bk_sb[:, do:do + 1])
                else:
                    nc.vector.tensor_copy(dst, psk[:])

            def proj_v(st):
                psv = ps_sc.tile([P, 512], F32, tag="sc")
                for ci in range(CO):
                    nc.tensor.matmul(
                        psv[:], vT_sb[:, ci, st * P:(st + 1) * P],
                        w_sb["wv"][:, ci, :],
                        start=(ci == 0), stop=(ci == CO - 1))
                dst = V_sb[:, st, :, 0:D_K]
                src = psv[:].rearrange("p (h e) -> p h e", h=N_HEADS)
                if qkv_bias:
                    nc.vector.tensor_tensor(
                        dst, src,
                        bv_bc[:].rearrange("p (h e) -> p h e", h=N_HEADS),
                        ALU.add)
                else:
                    nc.vector.tensor_copy(dst, src)

            # chunk 0 of K/Q up front; everything else rides inside attention
            for k0 in range(0, SK, 512):
                proj_k(0, k0)
            for q0 in range(0, SQ, 512):
                proj_q(0, q0)

            # ---- attention: head pairs, full 1024-q tiles ------------------
            # ctx_sb[d, do, q] pair-major (matches O-proj lhsT layout)
            ctx_sb = singles.tile([P, CO, SQ], BF16, tag="ctx")
            # softmax denominators, row j = (head, q-chunk) = 2h + c
            # row for (head h, q-chunk c): 32*(h//2) + 2*(h%2) + c, so a
            # pair's 4 rows start at a 32-aligned partition (engine rule)
            sums_sb = singles.tile([P, 512], F32, tag="sums")
            recip = singles.tile([P, 512], F32, tag="recip")
            recip_bf = singles.tile([P, 512], BF16, tag="recipbf")
            stage = singles.tile([1, 2 * N_HEADS, 512], BF16, tag="stg")

            def normalize_pair(pr):
                # 1/sums for pair pr, staged to partition-0 rows, broadcast
                # across psum partitions via 1-row PE outer products, applied
                hA_, hB_ = 2 * pr, 2 * pr + 1
                rows = slice(32 * pr, 32 * pr + 4)
                nc.vector.reciprocal(recip[rows, :], sums_sb[rows, :])
                nc.vector.tensor_copy(recip_bf[rows, :], recip[rows, :])
                for j in range(4):
                    nc.sync.dma_start(stage[:, 4 * pr + j, :],
                                      recip_bf[32 * pr + j:32 * pr + j + 1, :])
                for c in range(2):
                    rb = ps_sc.tile([P, 512], F32, tag="sc")
                    nc.tensor.matmul(rb[0:D_K, :], ones1[:],
                                     stage[:, 4 * pr + c, :],
                                     start=True, stop=True,
                                     tile_position=(0, 0),
                                     skip_group_check=True)
                    nc.tensor.matmul(rb[D_K:P, :], ones1[:],
                                     stage[:, 4 * pr + 2 + c, :],
                                     start=True, stop=True,
                                     tile_position=(0, D_K),
                                     skip_group_check=True)
                    cslice = ctx_sb[:, pr, c * 512:(c + 1) * 512]
                    nc.vector.tensor_tensor(cslice, cslice, rb[:], ALU.mult)

            def consume(ctxA, ctxB, pr, t, ptA, ptB):
                # DoubleRow ctx matmuls for k-tile pair t of head pair pr:
                # each instruction contracts both k-tiles of the pair
                first, last = t == 0, t == KT_TILES // 2 - 1
                for c in range(2):
                    q0 = c * 512
                    nc.tensor.matmul(
                        ctxA[:, c, :],
                        V_sb[:, 2 * t:2 * t + 2, 2 * pr, 0:D_K + 1],
                        ptA[:, :, q0:q0 + 512], start=first, stop=last,
                        perf_mode=DR)
                    nc.tensor.matmul(
                        ctxB[:, c, :],
                        V_sb[:, 2 * t:2 * t + 2, 2 * pr + 1, 0:D_K + 1],
                        ptB[:, :, q0:q0 + 512], start=first, stop=last,
                        perf_mode=DR)

            def finish_pair(pr, ctxA, ctxB):
                # unnormalized context -> SBUF bf16 (normalized later); head
                # B lands on partitions 64:128 (partition-base shift), then
                # denominators: psum row 64 -> SBUF -> DMA into sums_sb rows
                hA_, hB_ = 2 * pr, 2 * pr + 1
                nc.vector.tensor_copy(
                    ctx_sb[0:D_K, pr, :].rearrange("p (c q) -> p c q", c=2),
                    ctxA[0:D_K, :, :])
                nc.vector.tensor_copy(
                    ctx_sb[D_K:P, pr, :].rearrange("p (c q) -> p c q", c=2),
                    ctxB[0:D_K, :, :])
                sstA = small.tile([P, 2, 512], F32, tag="sstA")
                sstB = small.tile([P, 2, 512], F32, tag="sstB")
                nc.vector.tensor_copy(sstA[D_K:D_K + 1, :, :],
                                      ctxA[D_K:D_K + 1, :, :])
                nc.vector.tensor_copy(sstB[D_K:D_K + 1, :, :],
                                      ctxB[D_K:D_K + 1, :, :])
                for c in range(2):
                    r = 32 * pr
                    nc.sync.dma_start(sums_sb[r + c:r + c + 1, :],
                                      sstA[D_K:D_K + 1, c, :])
                    nc.sync.dma_start(sums_sb[r + 2 + c:r + 2 + c + 1, :],
                                      sstB[D_K:D_K + 1, c, :])

            # flat software pipeline over all 64 k-tiles: scores/exp of the
            # next k-tile always run ahead of the pending DR consume, so the
            # exp engine never stalls at pair boundaries.  Projections for V
            # and the next pair's K/Q chunks ride in unused PE slots.
            extra = {g: [] for g in range(NPAIR * KT_TILES)}
            for g in range(KT_TILES):
                extra[g].append(lambda g=g: proj_v(g))
            for p in range(1, NPAIR):
                base = KT_TILES * (p - 1)
                for idx, k0 in enumerate(range(0, SK, 512)):
                    extra[base + 2 + 2 * idx].append(
                        lambda p=p, k0=k0: proj_k(p, k0))
                for idx, q0 in enumerate(range(0, SQ, 512)):
                    extra[base + 10 + 2 * idx].append(
                        lambda p=p, q0=q0: proj_q(p, q0))
                extra[KT_TILES * p + 6].append(
                    lambda p=p: normalize_pair(p - 1))

            pending = None
            ptA = ptB = None
            ctxA = ctxB = None
            for g in range(NPAIR * KT_TILES):
                pr, kt = divmod(g, KT_TILES)
                t, i = divmod(kt, 2)
                for thunk in extra[g]:
                    thunk()
                if i == 0:
                    ptA = ptp.tile([P, 2, 1024], F8, tag="ptA")
                    ptB = ptp.tile([P, 2, 1024], F8, tag="ptB")
                k0 = kt * P
                sA = ps_sc.tile([P, 1024], F32, tag="sc")
                sB = ps_sc.tile([P, 1024], F32, tag="sc")
                for c in range(2):
                    q0 = c * 512
                    nc.tensor.matmul(
                        sA[:, q0:q0 + 512],
                        KT_sb[0:D_K, pr, k0:k0 + P],
                        QT_sb[0:D_K, pr, q0:q0 + 512],
                        start=True, stop=True)
                    nc.tensor.matmul(
                        sB[:, q0:q0 + 512],
                        KT_sb[D_K:P, pr, k0:k0 + P],
                        QT_sb[D_K:P, pr, q0:q0 + 512],
                        start=True, stop=True)
                nc.scalar.activation(ptA[:, i, :], sA[:], AF.Exp,
                                     bias=mb_sb[:, kt:kt + 1], scale=0.125)
                nc.scalar.activation(ptB[:, i, :], sB[:], AF.Exp,
                                     bias=mb_sb[:, kt:kt + 1], scale=0.125)
                if i == 1:
                    if pending is not None:
                        consume(*pending)
                    if t == 0:
                        # previous pair fully consumed: retire it, then
                        # rotate the ctx accumulators for this pair
                        if pending is not None:
                            finish_pair(pr - 1, pending[0], pending[1])
                        ctxA = ps_ctx.tile([D_K + 1, 2, 512], F32, tag="ctxA")
                        ctxB = ps_ctx.tile([D_K + 1, 2, 512], F32, tag="ctxB")
                    pending = (ctxA, ctxB, pr, t, ptA, ptB)
            consume(*pending)
            finish_pair(NPAIR - 1, pending[0], pending[1])
            normalize_pair(NPAIR - 1)

            # ---- output projection + residual + layernorm ------------------
            for qc in range(SQ // P):
                zps = ps_sc.tile([P, D_MODEL], F32, tag="sc")
                for do in range(CO):
                    nc.tensor.matmul(
                        zps[:], ctx_sb[:, do, qc * P:(qc + 1) * P],
                        w_sb["wo"][:, do, :],
                        start=(do == 0), stop=(do == CO - 1))
                z = stream.tile([P, D_MODEL], F32, tag="z")
                nc.vector.tensor_tensor(z[:], zps[:], qres_sb[:, qc, :], ALU.add)
                stats = small.tile([P, 6], F32, tag="stats")
                nc.vector.bn_stats(stats[:], z[:])
                mv = small.tile([P, 2], F32, tag="mv")
                nc.vector.bn_aggr(mv[:], stats[:])
                istd = small.tile([P, 1], F32, tag="istd")
                nc.scalar.activation(istd[:], mv[:, 1:2], AF.Sqrt,
                                     bias=eps_sb[:], scale=1.0)
                nc.vector.reciprocal(istd[:], istd[:])
                zo = stream.tile([P, D_MODEL], F32, tag="zo")
                nc.vector.tensor_scalar(zo[:], z[:], mv[:, 0:1], istd[:],
                                        ALU.subtract, ALU.mult)
                if ln_affine:
                    nc.vector.tensor_tensor(zo[:], zo[:], gamma_bc[:], ALU.mult)
                    nc.vector.tensor_tensor(zo[:], zo[:], beta_bc[:], ALU.add)
                nc.sync.dma_start(out[qc * P:(qc + 1) * P, :], zo[:])

    nc.compile()
    return nc


def _get_nc(qkv_bias: bool, ln_affine: bool):
    key = (qkv_bias, ln_affine)
    if key not in _NC_CACHE:
        _NC_CACHE[key] = _build(*key)
    return _NC_CACHE[key]


def prepare(query, key, value, key_mask, Wq, bq, Wk, bk, Wv, bv, Wo, bo,
            ln_gamma, ln_beta):
    """Host-side prep: returns (nc, in_maps) for the 8 cores."""
    query = np.asarray(query, dtype=np.float32)
    key = np.asarray(key, dtype=np.float32)
    value = np.asarray(value, dtype=np.float32)
    key_mask = np.asarray(key_mask)
    Wq = np.asarray(Wq, dtype=np.float32)
    Wk = np.asarray(Wk, dtype=np.float32)
    Wv = np.asarray(Wv, dtype=np.float32)
    Wo = np.asarray(Wo, dtype=np.float32)
    bq = np.asarray(bq, dtype=np.float32)
    bk = np.asarray(bk, dtype=np.float32)
    bv = np.asarray(bv, dtype=np.float32)
    bo = np.asarray(bo, dtype=np.float32)
    ln_gamma = np.asarray(ln_gamma, dtype=np.float32)
    ln_beta = np.asarray(ln_beta, dtype=np.float32)

    B, sq_full, dm = query.shape
    assert (B, sq_full, dm) == (4, 2048, 512), query.shape

    qkv_bias = bool(bq.any() or bk.any() or bv.any())
    ln_affine = bool((ln_gamma != 1.0).any() or ln_beta.any())
    nc = _get_nc(qkv_bias, ln_affine)

    bf = ml_dtypes.bfloat16
    wqT = np.ascontiguousarray(Wq.T).astype(bf)
    wkT = np.ascontiguousarray(Wk.T).astype(bf)
    wvT = np.ascontiguousarray(Wv.T).astype(bf)
    woT = np.ascontiguousarray(Wo.T).astype(bf)
    maskbias = np.where(key_mask, np.float32(0.0), np.float32(NEG))
    qres_full = query + bo[None, None, :]

    in_maps = []
    for core in range(N_CORES):
        b, half = divmod(core, 2)
        rows = slice(half * SQ, (half + 1) * SQ)
        m = {
            "qT": np.ascontiguousarray(query[b, rows].T).astype(bf),
            "kT": np.ascontiguousarray(key[b].T).astype(bf),
            "vT": np.ascontiguousarray(value[b].T).astype(bf),
            "qres": np.ascontiguousarray(qres_full[b, rows]),
            "wqT": wqT, "wkT": wkT, "wvT": wvT, "woT": woT,
            "maskbias": np.ascontiguousarray(maskbias[b]),
        }
        if qkv_bias:
            m["bq"], m["bk"], m["bv"] = bq, bk, bv
        if ln_affine:
            m["gamma"], m["beta"] = ln_gamma, ln_beta
        in_maps.append(m)
    return nc, in_maps


def kernel(**inputs):
    nc, in_maps = prepare(**inputs)
    B, sq_full, dm = 4, 2048, 512

    res = bass_utils.run_bass_kernel_spmd(nc, in_maps,
                                          core_ids=list(range(N_CORES)))
    out = np.empty((B, sq_full, dm), dtype=np.float32)
    for core in range(N_CORES):
        b, half = divmod(core, 2)
        out[b, half * SQ:(half + 1) * SQ] = res.results[core]["out"]
    return out

